# revision 52
# baseline (speedup 1.0000x reference)
"""BBoxHead (dense head + decode + class-aware NMS) for Trainium2, 8 cores.

Sharding: data-parallel over images. Core i processes image i (1000 RoIs,
padded to 1024). Dense-head weights are replicated. Decode + NMS are fully
per-image. Output [100, 6] per core is gathered to [8, 100, 6] on the host.

Per-core pipeline:
  1. Stream x [1024, 1024] in 8 row chunks; PE-transpose 128x128 blocks so
     features land on partitions; matmul against w=[w_cls|w_delta] (85 cols);
     bias added afterwards via a DMA-broadcast tile.
  2. Softmax score without per-class division: score = 1/sum(exp(l - lmax)).
     Argmax over classes via exact-equality + descending-iota reduce.
  3. delta2bbox decode + window clip, batched across chunk quarters
     (only the last quarter serializes behind the final matmul).
  4. NMS as greedy leader selection, unrolled as a nested-If cascade over
     NMS_LEVELS (this toolchain encodes one semaphore wait per instruction,
     which rules out Tile's barrier-based For_i back edge). Candidates stay
     in the partition-major [128, 8] layout; the three cross-partition steps
     per level ride tiny PE matmuls (global max via transpose+max, leader
     scalar gather via one-hot masked sums + ones-matmul, broadcast via
     rank-1 ones-matmul). Suppression (same-class IoU > 0.5), match_replace
     retirement and flag-masked emission are 128-lane-wide static ops. Exact
     for up to NMS_LEVELS detections per image; only levels < V
     (confident-box count) execute at runtime.

Toolchain notes: every fast instruction encoding here carries a single
semaphore wait, so producers are kept engine-uniform and one-off "probe"
reads make consumers observe foreign engine ticks early. The TileContext
tail is patched (_single_wait_drain_and_barrier) to split the kernel-tail
drain's wait list and skip the unsupported semaphore RANGE_CLEAR.
"""

import os
from contextlib import ExitStack

import numpy as np

import concourse.bass as bass
import concourse.mybir as mybir
import concourse.tile as tile
from concourse.tile import ScopedClock


def _single_wait_drain_and_barrier(self, tick_clock, wait_clock):
    """Replacement for TileContext._drain_and_barrier.

    This toolchain's walrus codegen encodes at most one semaphore wait per
    instruction and rejects the EVENT_SEMAPHORE_RANGE_CLEAR raw-ISA op, so:
    - split the kernel-tail drain's wait list into standalone wait_ge
      instructions (one wait each),
    - skip the semaphore range-clear (the runtime reinitializes semaphore
      state per NEFF execution).
    """
    nc = self.nc
    drain_inst = nc.sync.drain()
    wait_clock.add_sem_waits(
        drain_inst.ins, ScopedClock({None: tick_clock.global_clock})
    )
    si = drain_inst.ins.sync_info
    waits = list(si.on_wait) if si and si.on_wait else []
    if len(waits) > 1:
        handles = {h.num: h for h in self.sems.allocated().values()}
        drain_inst.ins.sync_info = mybir.SyncInfo(
            on_wait=[waits[0]], on_update=list(si.on_update or [])
        )
        for w in waits[1:]:
            nc.sync.wait_ge(handles[w.id], w.wait_value)
    nc.all_engine_barrier()
    popped = nc._tile_sem_poison_stack.pop()
    assert popped is self._sem_poison
    # (second barrier dropped: it only fenced the semaphore range-clear,
    # which this toolchain cannot encode and we skip)


tile.TileContext._drain_and_barrier = _single_wait_drain_and_barrier
from concourse.bass import ds
from concourse.bass_utils import run_bass_kernel_spmd
from concourse.expressions import smin
from concourse.masks import make_identity

F32 = mybir.dt.float32
I32 = mybir.dt.int32
U32 = mybir.dt.uint32
ALU = mybir.AluOpType
AF = mybir.ActivationFunctionType
AX = mybir.AxisListType

B = 8            # images == cores
R = 1000         # rois per image
RP = 1024        # padded rois per core
C = 1024         # feature dim
NCLS = 81        # classes (incl background 0)
NQ = 85          # 81 cls logits + 4 deltas
P = 128
NCHUNK = RP // P
KCHUNK = C // P
MIN_CONF = 0.05
CONF_BITS = int(np.float32(MIN_CONF).view(np.int32))
MAX_INST = 100
SCALE = 1024.0   # H == W == 1024
STDS = (0.1, 0.1, 0.2, 0.2)


def _emit(ctx: ExitStack, tc: tile.TileContext, x, rois, w, bvec, out):
    nc = tc.nc

    const = ctx.enter_context(tc.tile_pool(name="const", bufs=1))
    xpool = ctx.enter_context(tc.tile_pool(name="xin", bufs=8))
    xtpool = ctx.enter_context(tc.tile_pool(name="xt", bufs=8))
    ppool = ctx.enter_context(tc.tile_pool(name="ptr", bufs=4, space="PSUM"))
    lpool = ctx.enter_context(tc.tile_pool(name="plog", bufs=3, space="PSUM"))
    work = ctx.enter_context(tc.tile_pool(name="work", bufs=4))
    rowp = ctx.enter_context(tc.tile_pool(name="rows", bufs=1))
    loopp = ctx.enter_context(tc.tile_pool(name="loop", bufs=2))

    # ---- constants ----
    ident = const.tile([P, P], F32)
    make_identity(nc, ident)
    ones_col = const.tile([P, 1], F32)
    nc.gpsimd.memset(ones_col, 1.0)  # consumed by PE, which observes Pool via the warm-up
    iota_i = const.tile([P, NCLS], I32)
    nc.gpsimd.iota(iota_i, pattern=[[-1, NCLS]], base=NCLS, channel_multiplier=0)
    iota_f = const.tile([P, 1, NCLS], F32)
    nc.vector.tensor_copy(iota_f[:, 0, :], iota_i)  # int -> float convert
    stds = const.tile([P, 1, 4], F32)
    for j, v in enumerate(STDS):
        nc.gpsimd.memset(stds[:, :, j : j + 1], v)
    ones_row1 = const.tile([1, P], F32)
    nc.gpsimd.memset(ones_row1, 1.0)  # before the PE warm-up so PE observes it
    ZRm = const.tile([P, NCHUNK], F32)
    nc.gpsimd.memset(ZRm, 0.0)
    mrep = const.tile([P, NCHUNK], F32)
    nc.gpsimd.memset(mrep, 0.0)

    w_dma = const.tile([P, KCHUNK, NQ], F32)
    nc.sync.dma_start(out=w_dma, in_=w.rearrange("(c p) n -> p c n", p=P))
    w_sb = const.tile([P, KCHUNK, NQ], F32)
    nc.vector.tensor_copy(w_sb, w_dma)  # single-producer (DVE) for matmul rhs
    b_dma = const.tile([P, 1, NQ], F32)
    nc.gpsimd.dma_start(out=b_dma[:, 0, :], in_=bvec.to_broadcast((P, NQ)))
    b_bc = const.tile([P, 1, NQ], F32)
    nc.scalar.copy(b_bc, b_dma)  # ACT producer, same engine as LOG copies
    rp = const.tile([P, NCHUNK, 5], F32)
    nc.sync.dma_start(out=rp, in_=rois.rearrange("(c p) n -> p c n", p=P))

    LOG = rowp.tile([P, NCHUNK, NQ], F32)  # logits + deltas, SBUF copy
    QP = rowp.tile([P, 8, NCHUNK], F32)    # quantity-major: y1 x1 y2 x2 cls score area ms

    # PE warm-up: observe the Pool-made identity once, so every later
    # transpose carries a single (DMA) semaphore wait — the HW LDW struct
    # only has one wait slot.
    wps = ppool.tile([P, P], F32, tag="tp", name="warmup_ps")
    # Repeated self-transposes keep PE continuously busy through the first
    # x-chunk's DMA latency: the PE clock ungates (HAM p-state ramp) after
    # ~3us of sustained activity, so the real transposes/matmuls then run at
    # full rate. All writes hit the same dead tile -> PE-sequential, no sems.
    for _ in range(24):
        nc.tensor.transpose(wps, ident, ident)

    # ---- dense head: x @ [w_cls | w_delta] + b ----
    for c in range(NCHUNK):
        xc = xpool.tile([P, C], F32)
        nc.sync.dma_start(out=xc, in_=x[c * P : (c + 1) * P, :])
        xt = xtpool.tile([P, C], F32)
        for kb in range(KCHUNK):
            tp = ppool.tile([P, P], F32)
            nc.tensor.transpose(tp, xc[:, kb * P : (kb + 1) * P], ident)
            # one producer engine per chunk (alternating to balance DVE and
            # ACT): each consumer then sees exactly one producer semaphore,
            # which is all the fast instruction encodings can wait on.
            if c % 2 == 0:
                nc.vector.tensor_copy(xt[:, kb * P : (kb + 1) * P], tp)
            else:
                nc.scalar.copy(xt[:, kb * P : (kb + 1) * P], tp)
        pl = lpool.tile([P, NQ], F32)
        for kb in range(KCHUNK):
            nc.tensor.matmul(
                pl,
                xt[:, kb * P : (kb + 1) * P],
                w_sb[:, kb, :],
                start=(kb == 0),
                stop=(kb == KCHUNK - 1),
            )
        if c % 2 == 0:
            nc.vector.tensor_copy(LOG[:, c, :], pl)
        else:
            nc.scalar.copy(LOG[:, c, :], pl)

    _stage = int(os.environ.get("KDEBUG_STAGE", "4"))
    if _stage <= 1:
        return
    # ---- softmax score + argmax + bbox decode, in chunk quarters (only
    # the last quarter is serialized behind the final matmul) ----
    H = NCHUNK // 4
    lprb = work.tile([1, 4], F32, tag="lprb")
    for h in range(4):
        s = slice(h * H, (h + 1) * H)
        # observe the ACT-written LOG chunk once so the bias-add (and the
        # rest of the DVE chain) needs only one producer semaphore
        nc.vector.tensor_copy(
            lprb[0:1, h : h + 1], LOG[0:1, h * H + H - 1, 0:1]
        )
        nc.vector.tensor_add(
            LOG[:, s, :], LOG[:, s, :], b_bc.to_broadcast((P, H, NQ))
        )
        L = LOG[:, s, 0:NCLS]
        rmx = work.tile([P, H, 1], F32, tag="rmx")
        scr = work.tile([P, H, NCLS], F32, tag="scr")
        sm = work.tile([P, H, 1], F32, tag="sm")

        nc.vector.reduce_max(rmx, L, axis=AX.X)
        rb_ = rmx.to_broadcast((P, H, NCLS))
        nc.vector.tensor_sub(scr, L, rb_)
        nc.scalar.activation(scr, scr, AF.Exp)
        nc.vector.reduce_sum(sm, scr, axis=AX.X)
        nc.vector.reciprocal(QP[:, 5:6, s].rearrange("p one h -> p h one"), sm)  # score
        # argmax (first occurrence) == argmax of probs
        nc.vector.tensor_tensor(scr, L, rb_, op=ALU.is_equal)
        nc.vector.tensor_mul(scr, scr, iota_f.to_broadcast((P, H, NCLS)))
        nc.vector.reduce_max(rmx, scr, axis=AX.X)  # = NCLS - argmax
        nc.vector.tensor_scalar(
            QP[:, 4:5, s].rearrange("p one h -> p h one"),
            rmx, -1.0, float(NCLS), op0=ALU.mult, op1=ALU.add,
        )
        # masked score: score * [score >= conf] * [cls > 0]
        sc_ap = QP[:, 5:6, s].rearrange("p one h -> p h one")
        cls_ap = QP[:, 4:5, s].rearrange("p one h -> p h one")
        ms_ap = QP[:, 7:8, s].rearrange("p one h -> p h one")
        nc.vector.scalar_tensor_tensor(
            sm, sc_ap, MIN_CONF, sc_ap, op0=ALU.is_ge, op1=ALU.mult
        )
        nc.vector.scalar_tensor_tensor(
            ms_ap, cls_ap, 0.5, sm, op0=ALU.is_gt, op1=ALU.mult
        )

        # decode boxes
        rb = work.tile([P, H, 4], F32, tag="rb")
        dd = work.tile([P, H, 4], F32, tag="dd")
        hw = work.tile([P, H, 2], F32, tag="hw")
        t1 = work.tile([P, H, 2], F32, tag="t1")
        ctr = work.tile([P, H, 2], F32, tag="ctr")
        ex = work.tile([P, H, 2], F32, tag="ex")
        nc.vector.tensor_scalar_mul(rb, rp[:, s, 1:5], SCALE)
        nc.vector.tensor_mul(dd, LOG[:, s, NCLS : NCLS + 4], stds.to_broadcast((P, H, 4)))
        nc.vector.tensor_sub(hw, rb[:, :, 2:4], rb[:, :, 0:2])
        nc.vector.tensor_mul(t1, dd[:, :, 0:2], hw)
        nc.vector.scalar_tensor_tensor(t1, hw, 0.5, t1, op0=ALU.mult, op1=ALU.add)
        nc.vector.tensor_add(ctr, rb[:, :, 0:2], t1)
        nc.scalar.activation(ex, dd[:, :, 2:4], AF.Exp)
        nc.vector.tensor_mul(hw, hw, ex)
        nc.vector.tensor_scalar_mul(hw, hw, 0.5)
        tl_ap = QP[:, 0:2, s].rearrange("p q h -> p h q")
        br_ap = QP[:, 2:4, s].rearrange("p q h -> p h q")
        box_ap = QP[:, 0:4, s].rearrange("p q h -> p h q")
        ar_ap = QP[:, 6:7, s].rearrange("p one h -> p h one")
        nc.vector.tensor_sub(tl_ap, ctr, hw)
        nc.vector.tensor_add(br_ap, ctr, hw)
        nc.vector.tensor_scalar_max(box_ap, box_ap, 0.0)
        nc.vector.tensor_scalar_min(box_ap, box_ap, SCALE)
        nc.vector.tensor_sub(t1, br_ap, tl_ap)
        nc.vector.tensor_mul(ar_ap, t1[:, :, 0:1], t1[:, :, 1:2])

    # padded rois (rows 1000..1023 = chunk 7 partitions 104..127) can never
    # be valid: zero their masked score via a partition-index mask.
    pidx_i = const.tile([P, 1], I32)
    nc.gpsimd.iota(pidx_i, pattern=[[0, 1]], base=0, channel_multiplier=1)
    pidx_f = const.tile([P, 1], F32)
    nc.vector.tensor_copy(pidx_f, pidx_i)
    padm = const.tile([P, 1], F32)
    nc.vector.tensor_scalar(
        padm, pidx_f, float(R - 1 - (NCHUNK - 1) * P), None, op0=ALU.is_le
    )
    nc.vector.tensor_mul(
        QP[:, 7, NCHUNK - 1 : NCHUNK], QP[:, 7, NCHUNK - 1 : NCHUNK], padm
    )

    if _stage <= 2:
        return
    # ---- count of confident boxes -> register ----
    v01 = work.tile([P, NCHUNK], F32, tag="v01")
    nc.vector.tensor_scalar(v01, QP[:, 7, :], MIN_CONF, None, op0=ALU.is_ge)
    rsum = work.tile([P, 1], F32, tag="rsum")
    nc.vector.reduce_sum(rsum, v01, axis=AX.X)
    pcnt = ppool.tile([1, 1], F32, tag="qrow_ps", name="pcnt", bufs=1)
    nc.tensor.matmul(pcnt, ones_col, rsum, start=True, stop=True)
    vcnt_f = work.tile([1, 1], F32, tag="vcf")
    nc.vector.tensor_copy(vcnt_f, pcnt)
    vcnt_i = work.tile([1, 1], I32, tag="vci")
    nc.vector.tensor_copy(vcnt_i, vcnt_f)

    if _stage <= 3:
        return
    # Candidates stay in the partition-major QP2 snapshot [128, q, 8].
    # Cross-partition steps (global max, leader gather, broadcast) go through
    # tiny PE matmuls; everything else runs 128-lanes-wide.
    QP2 = rowp.tile([P, 8, NCHUNK], F32)
    nc.vector.tensor_copy(QP2, QP)  # single producer tick for the cascade
    MSm = rowp.tile([P, NCHUNK], F32)
    nc.vector.tensor_copy(MSm, QP2[:, 7, :])
    SOUT = rowp.tile([1, 6 * MAX_INST], F32)
    nc.vector.memset(SOUT, 0.0)
    prb = rowp.tile([1, 4], F32)
    nc.vector.tensor_copy(prb[0:1, 0:1], ZRm[0:1, 0:1])
    nc.vector.tensor_copy(prb[0:1, 1:2], mrep[0:1, 0:1])
    nc.vector.tensor_copy(prb[0:1, 2:3], stds[:1, 0, 0:1])

    # ---- greedy NMS leader cascade (see module docstring) ----
    vregs = nc.alloc_registers("vcnt_r", bass.OrderedSet(list(mybir.ALL_ENGINES)))
    for reg in vregs:
        nc.reg_load(reg, vcnt_i[0:1, 0:1])
    vval = nc.snap(vregs)
    with ExitStack() as lvl_stack:
        for lvl in range(NMS_LEVELS):
            lvl_stack.enter_context(tc.If(vval > lvl, name=f"nms{lvl}"))
            m8p = loopp.tile([P, 8], F32, tag="m8p", name=f"m8p_{lvl}")
            nc.vector.max(m8p, MSm)
            pmt = ppool.tile([1, P], F32, tag="qrow_ps", name=f"pmt_{lvl}", bufs=1)
            nc.tensor.transpose(pmt, m8p[:, 0:1], ident)
            smr = loopp.tile([1, P], F32, tag="smr", name=f"smr_{lvl}")
            nc.vector.tensor_copy(smr, pmt)
            m8g = loopp.tile([1, 8], F32, tag="m8g", name=f"m8g_{lvl}")
            nc.vector.max(m8g, smr)
            ms0 = m8g[0:1, 0:1]
            flg = loopp.tile([1, 1], F32, tag="flg", name=f"flg_{lvl}")
            nc.vector.tensor_scalar(flg, ms0, MIN_CONF, None, op0=ALU.is_ge)
            # broadcast the max to all partitions (-> one-hot + retirement key)
            pmb = ppool.tile([P, 1], F32, tag="qrow_ps", name=f"pmb_{lvl}", bufs=1)
            nc.tensor.matmul(pmb, ones_row1, ms0, start=True, stop=True)
            nc.vector.tensor_copy(mrep[:, 0:1], pmb)
            msk = loopp.tile([P, NCHUNK], F32, tag="msk", name=f"msk_{lvl}")
            nc.vector.tensor_scalar(msk, MSm, mrep[:, 0:1], None, op0=ALU.is_equal)
            # gather leader scalars: per-partition masked sums, then a
            # ones-matmul folds partitions (exact: one nonzero term)
            junkm = loopp.tile([P, NCHUNK], F32, tag="junkm", name=f"junkm_{lvl}")
            pp = loopp.tile([P, 6], F32, tag="pp", name=f"pp_{lvl}")
            for q, qi in enumerate((0, 1, 2, 3, 4, 6)):  # y1 x1 y2 x2 cls area
                nc.vector.scalar_tensor_tensor(
                    junkm, QP2[:, qi, :], 1.0, msk, op0=ALU.mult, op1=ALU.mult,
                    accum_out=pp[:, q : q + 1],
                )
            pld = ppool.tile([1, 6], F32, tag="qrow_ps", name=f"pld_{lvl}", bufs=1)
            nc.tensor.matmul(pld, ones_col, pp, start=True, stop=True)
            LEAD = loopp.tile([1, 6], F32, tag="lead", name=f"lead_{lvl}")
            nc.vector.tensor_copy(LEAD, pld)
            plb = ppool.tile([P, 6], F32, tag="qrow_ps", name=f"plb_{lvl}", bufs=1)
            nc.tensor.matmul(plb, ones_row1, LEAD, start=True, stop=True)
            LB = loopp.tile([P, 6], F32, tag="lb", name=f"lb_{lvl}")
            nc.vector.tensor_copy(LB, plb)
            y1s, x1s = LB[:, 0:1], LB[:, 1:2]
            y2s, x2s = LB[:, 2:3], LB[:, 3:4]
            crs, ars = LB[:, 4:5], LB[:, 5:6]
            ta = loopp.tile([P, NCHUNK], F32, tag="ta", name=f"ta_{lvl}")
            tb = loopp.tile([P, NCHUNK], F32, tag="tb", name=f"tb_{lvl}")
            tcm = loopp.tile([P, NCHUNK], mybir.dt.uint8, tag="tc", name=f"tc_{lvl}")
            nc.vector.tensor_scalar(ta, QP2[:, 0, :], y1s, None, op0=ALU.max)
            nc.vector.scalar_tensor_tensor(
                ta, QP2[:, 2, :], y2s, ta, op0=ALU.min, op1=ALU.subtract
            )  # ih
            nc.vector.tensor_scalar(tb, QP2[:, 1, :], x1s, None, op0=ALU.max)
            nc.vector.scalar_tensor_tensor(
                tb, QP2[:, 3, :], x2s, tb, op0=ALU.min, op1=ALU.subtract
            )  # iw
            nc.vector.tensor_scalar_max(tb, tb, 0.0)
            nc.vector.scalar_tensor_tensor(
                ta, ta, 0.0, tb, op0=ALU.max, op1=ALU.mult
            )  # inter
            nc.vector.scalar_tensor_tensor(
                tb, QP2[:, 6, :], ars, ta, op0=ALU.add, op1=ALU.subtract
            )  # union
            nc.vector.tensor_scalar_max(tb, tb, 1e-8)
            nc.vector.scalar_tensor_tensor(
                tcm, ta, 2.0, tb, op0=ALU.mult, op1=ALU.is_gt
            )  # iou > 0.5
            nc.vector.scalar_tensor_tensor(
                tcm, QP2[:, 4, :], crs, tcm, op0=ALU.is_equal, op1=ALU.logical_and
            )  # same class & iou > 0.5 (no-op when flag=0: zero leader box)
            nc.vector.copy_predicated(MSm, tcm, ZRm)
            nc.vector.match_replace(
                out=MSm, in_to_replace=mrep.rearrange("p (a b) -> p a b", a=1)[:, 0, :],
                in_values=MSm, imm_value=0.0,
            )  # retire the leader even if its own iou row missed it
            for q in range(6):  # box(4), cls, score(=max)
                src_ap = LEAD[0:1, q : q + 1] if q < 5 else ms0
                nc.vector.tensor_scalar(
                    SOUT[0:1, q * MAX_INST + lvl : q * MAX_INST + lvl + 1],
                    src_ap,
                    flg[0:1, 0:1],
                    None,
                    op0=ALU.mult,
                )

    # Pool-side probe read of the full staging row: the Pool sequencer then
    # observes DVE's final tick, so the out-DMA carries a single queue wait.
    sprobe = rowp.tile([1, 1], F32)
    nc.gpsimd.tensor_copy(
        sprobe, SOUT[0:1, 5 * MAX_INST + NMS_LEVELS - 1 : 5 * MAX_INST + NMS_LEVELS]
    )  # statically-last SOUT write: Pool observes DVE's final tick
    nc.gpsimd.dma_start(
        out=out.rearrange("r q -> q r"),
        in_=SOUT.rearrange("one (q r) -> one q r", q=6),
    )


_MODULE = None


def _get_module():
    global _MODULE
    if _MODULE is None:
        nc = bass.Bass()
        x = nc.declare_dram_parameter("x", [RP, C], F32, isOutput=False)
        rois = nc.declare_dram_parameter("rois", [RP, 5], F32, isOutput=False)
        w = nc.declare_dram_parameter("w", [C, NQ], F32, isOutput=False)
        bvec = nc.declare_dram_parameter("bvec", [1, NQ], F32, isOutput=False)
        out = nc.declare_dram_parameter("out", [MAX_INST, 6], F32, isOutput=True)
        with tile.TileContext(nc) as tc:
            with ExitStack() as ctx:
                _emit(ctx, tc, x[:], rois[:], w[:], bvec[:], out[:])
        _MODULE = nc
    return _MODULE


def make_in_maps(x, w_cls, b_cls, w_delta, b_delta, rois):
    x = np.ascontiguousarray(np.asarray(x, np.float32))
    rois = np.ascontiguousarray(np.asarray(rois, np.float32))
    w_full = np.ascontiguousarray(
        np.concatenate(
            [np.asarray(w_cls, np.float32), np.asarray(w_delta, np.float32)], axis=1
        )
    )
    b_full = np.concatenate(
        [np.asarray(b_cls, np.float32), np.asarray(b_delta, np.float32)]
    ).reshape(1, NQ)
    in_maps = []
    for i in range(B):
        xp = np.zeros((RP, C), np.float32)
        xp[:R] = x[i * R : (i + 1) * R]
        rp_ = np.zeros((RP, 5), np.float32)
        rp_[:R] = rois[i * R : (i + 1) * R]
        in_maps.append({"x": xp, "rois": rp_, "w": w_full, "bvec": b_full})
    return in_maps


LAST_RESULTS = None


def kernel(x, w_cls, b_cls, w_delta, b_delta, rois):
    global LAST_RESULTS
    nc = _get_module()
    in_maps = make_in_maps(x, w_cls, b_cls, w_delta, b_delta, rois)
    trace = bool(int(os.environ.get("KBENCH_TRACE", "0")))
    res = run_bass_kernel_spmd(nc, in_maps, list(range(B)), trace=trace)
    LAST_RESULTS = res
    return np.stack([res.results[i]["out"] for i in range(B)]).astype(np.float32)
